# revision 3
# baseline (speedup 1.0000x reference)
"""Distributed causal MHA for Trainium2 (8 NeuronCores).

Problem: B=2, S=2048, D=1024, H=16, A=64 causal attention block.

Sharding: tensor-parallel over heads. Each core owns 2 heads end-to-end
(QKV projection + attention), then an AllToAll exchanges z from
head-sharded to sequence-sharded, and each core computes the output
projection for its 512-token shard (contraction over all 16 heads).
Host concatenates the 8 token shards. No reduction on host.

Layouts (per core):
  xT   [D, T=4096]    residual transposed, tokens b-major (bf16)
  wq/wk/wv [D, 128]   2 local heads packed (h*64+a) (bf16)
  woR  [8, 128, D]    full Wo as (pair j, (h,a), d) (bf16)
  qT/kT [128, T]      (h*64+a) on partitions
  v    [128p=t, 32tc, 2h, 65]  v augmented with ones column (denominator trick)
  zT   [128, T]       attention out, head-packed
  out  [D, 512]       output-projection result for this core's token shard (f32)
"""

import numpy as np
import ml_dtypes

import concourse.bass as bass
import concourse.mybir as mybir
import concourse.tile as tile
from concourse import bacc
from concourse.bass import ts, ds
from concourse.bass_utils import run_bass_kernel_spmd

AF = mybir.ActivationFunctionType
F32 = mybir.dt.float32
BF16 = mybir.dt.bfloat16

B, S, D, H, A = 2, 2048, 1024, 16, 64
NCORES = 8
HPC = H // NCORES          # 2 heads per core
T = B * S                  # 4096 tokens
P = 128
DK = D // P                # 8 contraction chunks
NT = 512                   # free-dim tile for projections / attention
TSH = T // NCORES          # 512 tokens per core shard (output)
SCALE = 1.0 / np.sqrt(A)
NEG = -1.0e9

_CACHE = {}


def _build():
    nc = bacc.Bacc("TRN2", target_bir_lowering=False, debug=False,
                   num_devices=NCORES)
    xT = nc.dram_tensor("xT", [D, T], BF16, kind="ExternalInput")
    wq = nc.dram_tensor("wq", [D, 2 * A], BF16, kind="ExternalInput")
    wk = nc.dram_tensor("wk", [D, 2 * A], BF16, kind="ExternalInput")
    wv = nc.dram_tensor("wv", [D, 2 * A], BF16, kind="ExternalInput")
    woR = nc.dram_tensor("woR", [NCORES, 2 * A, D], BF16, kind="ExternalInput")
    bqd = nc.dram_tensor("bqd", [2 * A, 1], F32, kind="ExternalInput")
    bkd = nc.dram_tensor("bkd", [2 * A, 1], F32, kind="ExternalInput")
    bvd = nc.dram_tensor("bvd", [1, 2 * A], F32, kind="ExternalInput")
    bod = nc.dram_tensor("bod", [P, DK], F32, kind="ExternalInput")
    mkd = nc.dram_tensor("mkd", [4, P, NT], F32, kind="ExternalInput")
    out = nc.dram_tensor("out", [D, TSH], F32, kind="ExternalOutput")

    with tile.TileContext(nc) as tc:
        with tc.tile_pool(name="big", bufs=1) as big, \
             tc.tile_pool(name="work", bufs=3) as work, \
             tc.tile_pool(name="mm_ps", bufs=2, space="PSUM") as mm_ps, \
             tc.tile_pool(name="sc_ps", bufs=2, space="PSUM") as sc_ps, \
             tc.tile_pool(name="z_ps", bufs=2, space="PSUM") as z_ps, \
             tc.tile_pool(name="dram", bufs=1, space="DRAM") as dram:

            # ---- load inputs to SBUF ----
            x_sb = big.tile([P, DK, T], BF16)
            nc.sync.dma_start(x_sb[:], xT.ap().rearrange("(ko p) t -> p ko t", p=P))
            wq_sb = big.tile([P, DK, 2 * A], BF16)
            nc.sync.dma_start(wq_sb[:], wq.ap().rearrange("(ko p) m -> p ko m", p=P))
            wk_sb = big.tile([P, DK, 2 * A], BF16)
            nc.sync.dma_start(wk_sb[:], wk.ap().rearrange("(ko p) m -> p ko m", p=P))
            wv_sb = big.tile([P, DK, 2 * A], BF16)
            nc.sync.dma_start(wv_sb[:], wv.ap().rearrange("(ko p) m -> p ko m", p=P))
            wo_sb = big.tile([P, NCORES, DK, P], BF16)
            nc.sync.dma_start(
                wo_sb[:], woR.ap().rearrange("j p (mo mi) -> p j mo mi", mi=P))
            bq_sb = big.tile([2 * A, 1], F32)
            nc.sync.dma_start(bq_sb[:], bqd[:])
            bk_sb = big.tile([2 * A, 1], F32)
            nc.sync.dma_start(bk_sb[:], bkd[:])
            bv_sb = big.tile([1, 2 * A], F32)
            nc.sync.dma_start(bv_sb[:], bvd[:])
            bo_sb = big.tile([P, DK], F32)
            nc.sync.dma_start(bo_sb[:], bod[:])
            mask_sb = big.tile([P, 4, NT], F32)
            nc.sync.dma_start(mask_sb[:], mkd.ap().rearrange("m p s -> p m s"))
            ones1 = big.tile([1, A], F32)
            nc.any.memset(ones1[:], 1.0)
            onesc = big.tile([1, P], F32)
            nc.any.memset(onesc[:], 1.0)

            # ---- persistent activations ----
            qT_sb = big.tile([P, T], BF16)     # (h*64+a, token)
            kT_sb = big.tile([P, T], BF16)
            v_sb = big.tile([P, T // P, HPC, A + 1], BF16)  # (t_in, t_chunk, h, a+den)
            nc.any.memset(v_sb[:, :, :, A], 1.0)
            zT_sb = big.tile([P, T], BF16)

            # ---- projections: qT, kT ----
            for dst, w_sb, b_sb in ((qT_sb, wq_sb, bq_sb), (kT_sb, wk_sb, bk_sb)):
                for nt in range(T // NT):
                    ps = mm_ps.tile([P, NT], F32, tag="mm")
                    for ko in range(DK):
                        nc.tensor.matmul(ps[:], w_sb[:, ko, :],
                                         x_sb[:, ko, ts(nt, NT)],
                                         start=(ko == 0), stop=(ko == DK - 1))
                    nc.scalar.activation(dst[:, ts(nt, NT)], ps[:],
                                         AF.Identity, bias=b_sb[:])

            # ---- projection: v (tokens on partitions) ----
            for tc_i in range(T // P):
                ps = mm_ps.tile([P, NT], F32, tag="mm")
                pv = ps[:, 0:2 * A]
                for ko in range(DK):
                    nc.tensor.matmul(pv, x_sb[:, ko, ts(tc_i, P)], wv_sb[:, ko, :],
                                     start=(ko == 0), stop=False)
                # += ones_col.T @ bv  (adds bias to every token row)
                nc.tensor.matmul(pv, onesc[:], bv_sb[:], start=False, stop=True)
                nc.vector.tensor_copy(out=v_sb[:, tc_i, :, 0:A], in_=pv)

            # ---- attention per (b, h) ----
            for b in range(B):
                base = b * S
                for h in range(HPC):
                    hs = ds(h * A, A)
                    for st in range(S // NT):
                        zp = z_ps.tile([A + 1, NT], F32, tag="z")
                        nblk = 4 * st + 4
                        for tb in range(nblk):
                            sp = sc_ps.tile([P, NT], F32, tag="sc")
                            nc.tensor.matmul(
                                sp[:],
                                kT_sb[hs, base + 128 * tb:base + 128 * (tb + 1)],
                                qT_sb[hs, base + NT * st:base + NT * (st + 1)],
                                start=True, stop=True)
                            m = tb - 4 * st
                            p_sb = work.tile([P, NT], BF16, tag="p")
                            if m >= 0:
                                msk = work.tile([P, NT], F32, tag="msk")
                                nc.vector.tensor_tensor(
                                    out=msk[:], in0=sp[:], in1=mask_sb[:, m, :],
                                    op=mybir.AluOpType.add)
                                nc.scalar.activation(p_sb[:], msk[:], AF.Exp,
                                                     scale=SCALE)
                            else:
                                nc.scalar.activation(p_sb[:], sp[:], AF.Exp,
                                                     scale=SCALE)
                            nc.tensor.matmul(
                                zp[:], v_sb[:, b * (S // P) + tb, h, :], p_sb[:],
                                start=(tb == 0), stop=(tb == nblk - 1))
                        # normalize: z / denom (denom in row A of zp)
                        den = work.tile([1, NT], F32, tag="den")
                        nc.vector.tensor_copy(out=den[:], in_=zp[A:A + 1, :])
                        rec = work.tile([1, NT], F32, tag="rec")
                        nc.vector.reciprocal(rec[:], den[:])
                        bc = sc_ps.tile([P, NT], F32, tag="sc")
                        nc.tensor.matmul(bc[0:A, :], ones1[:], rec[:],
                                         start=True, stop=True)
                        bc_sb = work.tile([A, NT], F32, tag="bc")
                        nc.scalar.activation(bc_sb[:], bc[0:A, :], AF.Identity)
                        nc.vector.tensor_tensor(
                            out=zT_sb[hs, base + NT * st:base + NT * (st + 1)],
                            in0=zp[0:A, :], in1=bc_sb[:],
                            op=mybir.AluOpType.mult)

            # ---- AllToAll: head-sharded -> token-sharded ----
            a2a_in = dram.tile([NCORES, P, TSH], BF16)
            a2a_out = dram.tile([NCORES, P, TSH], BF16)
            for j in range(NCORES):
                nc.sync.dma_start(a2a_in[j], zT_sb[:, ts(j, TSH)])
            nc.gpsimd.collective_compute(
                "AllToAll", mybir.AluOpType.bypass,
                replica_groups=[list(range(NCORES))],
                ins=[a2a_in.opt()], outs=[a2a_out.opt()])
            zr_sb = big.tile([P, NCORES, TSH], BF16)
            for j in range(NCORES):
                nc.sync.dma_start(zr_sb[:, j, :], a2a_out[j])

            # ---- output projection for this core's token shard ----
            for mo in range(DK):
                ps = mm_ps.tile([P, NT], F32, tag="mm")
                po = ps[:, 0:TSH]
                for j in range(NCORES):
                    nc.tensor.matmul(po, wo_sb[:, j, mo, :], zr_sb[:, j, :],
                                     start=(j == 0), stop=(j == NCORES - 1))
                osb = work.tile([P, TSH], F32, tag="o")
                nc.scalar.activation(osb[:], po, AF.Identity,
                                     bias=bo_sb[:, mo:mo + 1])
                nc.sync.dma_start(out.ap()[ts(mo, P), :], osb[:])

    nc.compile()
    return nc


def _prep_inputs(residual, Wq, Wk, Wv, Wo, bq, bk, bv, bo):
    bf = ml_dtypes.bfloat16
    residual = np.asarray(residual, np.float32)
    xT = np.ascontiguousarray(residual.reshape(T, D).T).astype(bf)
    woR = np.ascontiguousarray(
        np.asarray(Wo, np.float32).reshape(NCORES, 2 * A, D)).astype(bf)
    boR = np.ascontiguousarray(np.asarray(bo, np.float32).reshape(DK, P).T)
    # causal masks for the 4 diagonal sub-blocks
    tt = np.arange(P)[:, None]
    ss = np.arange(NT)[None, :]
    mk = np.stack([np.where(128 * m + tt <= ss, 0.0, NEG)
                   for m in range(4)]).astype(np.float32)
    in_maps = []
    for i in range(NCORES):
        hsl = slice(HPC * i, HPC * (i + 1))
        wqi = np.ascontiguousarray(
            np.asarray(Wq, np.float32)[hsl].transpose(1, 0, 2).reshape(D, 2 * A)
        ).astype(bf)
        wki = np.ascontiguousarray(
            np.asarray(Wk, np.float32)[hsl].transpose(1, 0, 2).reshape(D, 2 * A)
        ).astype(bf)
        wvi = np.ascontiguousarray(
            np.asarray(Wv, np.float32)[hsl].transpose(1, 0, 2).reshape(D, 2 * A)
        ).astype(bf)
        in_maps.append({
            "xT": xT, "wq": wqi, "wk": wki, "wv": wvi, "woR": woR,
            "bqd": np.asarray(bq, np.float32)[hsl].reshape(2 * A, 1),
            "bkd": np.asarray(bk, np.float32)[hsl].reshape(2 * A, 1),
            "bvd": np.asarray(bv, np.float32)[hsl].reshape(1, 2 * A),
            "bod": boR, "mkd": mk,
        })
    return in_maps


def kernel(residual, Wq, Wk, Wv, Wo, bq, bk, bv, bo, _trace=False):
    if "nc" not in _CACHE:
        _CACHE["nc"] = _build()
    nc = _CACHE["nc"]
    in_maps = _prep_inputs(residual, Wq, Wk, Wv, Wo, bq, bk, bv, bo)
    res = run_bass_kernel_spmd(nc, in_maps, core_ids=list(range(NCORES)),
                               trace=_trace)
    _CACHE["last_result"] = res
    outT = np.concatenate(
        [res.results[i]["out"].T for i in range(NCORES)], axis=0)
    return np.ascontiguousarray(outT.reshape(B, S, D))


# revision 10
# speedup vs baseline: 2.9351x; 2.9351x over previous
"""Distributed causal MHA for Trainium2 (8 NeuronCores).

Problem: B=2, S=2048, D=1024, H=16, A=64 causal attention block.

Sharding: tensor-parallel over heads. Each core owns 2 heads end-to-end
(QKV projection + attention), then an AllToAll exchanges z from
head-sharded to sequence-sharded, and each core computes the output
projection for its 512-token shard (contraction over all 16 heads).
Host concatenates the 8 token shards. No reduction on host.

Layouts (per core):
  xT   [D, T=4096]    residual transposed, tokens b-major (bf16)
  wq/wk/wv [D, 128]   2 local heads packed (h*64+a) (bf16)
  woR  [8, 128, D]    full Wo as (pair j, (h,a), d) (bf16)
  qT/kT [128, T]      (h*64+a) on partitions
  v    [128p=t, 32tc, 2h, 65]  v augmented with ones column (denominator trick)
  zT   [128, T]       attention out, head-packed
  out  [D, 512]       output-projection result for this core's token shard (f32)
"""

import numpy as np
import ml_dtypes

import concourse.bass as bass
import concourse.mybir as mybir
import concourse.tile as tile
from concourse import bacc
from concourse.bass import ts, ds
from concourse.bass_utils import run_bass_kernel_spmd

AF = mybir.ActivationFunctionType
F32 = mybir.dt.float32
BF16 = mybir.dt.bfloat16

B, S, D, H, A = 2, 2048, 1024, 16, 64
NCORES = 8
HPC = H // NCORES          # 2 heads per core
T = B * S                  # 4096 tokens
P = 128
DK = D // P                # 8 contraction chunks
NT = 512                   # free-dim tile for projections / attention
TSH = T // NCORES          # 512 tokens per core shard (output)
SCALE = 1.0 / np.sqrt(A)
NEG = -1.0e9

_CACHE = {}


def _build(collective=True):
    nc = bacc.Bacc("TRN2", target_bir_lowering=False, debug=False,
                   num_devices=NCORES)
    xT = nc.dram_tensor("xT", [D, T], BF16, kind="ExternalInput")
    wq = nc.dram_tensor("wq", [D, 2 * A], BF16, kind="ExternalInput")
    wk = nc.dram_tensor("wk", [D, 2 * A], BF16, kind="ExternalInput")
    wv = nc.dram_tensor("wv", [D, 2 * A], BF16, kind="ExternalInput")
    woR = nc.dram_tensor("woR", [NCORES, 2 * A, D], BF16, kind="ExternalInput")
    bqd = nc.dram_tensor("bqd", [2 * A, 1], F32, kind="ExternalInput")
    bkd = nc.dram_tensor("bkd", [2 * A, 1], F32, kind="ExternalInput")
    bvd = nc.dram_tensor("bvd", [1, 2 * A], F32, kind="ExternalInput")
    bod = nc.dram_tensor("bod", [P, DK], F32, kind="ExternalInput")
    mkd = nc.dram_tensor("mkd", [4, P, NT], BF16, kind="ExternalInput")
    out = nc.dram_tensor("out", [D, TSH], F32, kind="ExternalOutput")

    with tile.TileContext(nc) as tc:
        with tc.tile_pool(name="big", bufs=1) as big, \
             tc.tile_pool(name="work", bufs=3) as work, \
             tc.tile_pool(name="mm_ps", bufs=2, space="PSUM") as mm_ps, \
             tc.tile_pool(name="sc_ps", bufs=4, space="PSUM") as sc_ps, \
             tc.tile_pool(name="z_ps", bufs=2, space="PSUM") as z_ps, \
             tc.tile_pool(name="dram", bufs=1, space="DRAM") as dram:

            # ---- load inputs to SBUF (weights first, x in chunks) ----
            wq_sb = big.tile([P, DK, 2 * A], BF16)
            nc.sync.dma_start(wq_sb[:], wq.ap().rearrange("(ko p) m -> p ko m", p=P))
            wk_sb = big.tile([P, DK, 2 * A], BF16)
            nc.sync.dma_start(wk_sb[:], wk.ap().rearrange("(ko p) m -> p ko m", p=P))
            wv_sb = big.tile([P, DK, 2 * A], BF16)
            nc.sync.dma_start(wv_sb[:], wv.ap().rearrange("(ko p) m -> p ko m", p=P))
            wo_sb = big.tile([P, NCORES, DK, P], BF16)
            nc.sync.dma_start(
                wo_sb[:], woR.ap().rearrange("j p (mo mi) -> p j mo mi", mi=P))
            bq_sb = big.tile([2 * A, 1], F32)
            nc.sync.dma_start(bq_sb[:], bqd[:])
            bk_sb = big.tile([2 * A, 1], F32)
            nc.sync.dma_start(bk_sb[:], bkd[:])
            bv_sb = big.tile([1, 2 * A], F32)
            nc.sync.dma_start(bv_sb[:], bvd[:])
            bo_sb = big.tile([P, DK], F32)
            nc.sync.dma_start(bo_sb[:], bod[:])
            mask_sb = big.tile([P, 4, NT], BF16)
            nc.sync.dma_start(mask_sb[:], mkd.ap().rearrange("m p s -> p m s"))
            x_sb = big.tile([P, DK, T], BF16)
            for tq in range(4):
                for ko in range(DK):
                    nc.sync.dma_start(x_sb[:, ko, ts(tq, T // 4)],
                                      xT.ap()[ts(ko, P), ts(tq, T // 4)])
            ones1 = big.tile([1, A], F32)
            nc.any.memset(ones1[:], 1.0)
            onesc = big.tile([1, P], F32)
            nc.any.memset(onesc[:], 1.0)

            # ---- persistent activations ----
            qT_sb = big.tile([P, T], BF16)     # (h*64+a, token)
            kT_sb = big.tile([P, T], BF16)
            v_sb = big.tile([P, T // P, HPC, A + 1], BF16)  # (t_in, t_chunk, h, a+den)
            nc.any.memset(v_sb[:, :, :, A], 1.0)
            zT_sb = big.tile([P, T], BF16)

            # ---- projections: qT, kT ----
            for dst, w_sb, b_sb in ((qT_sb, wq_sb, bq_sb), (kT_sb, wk_sb, bk_sb)):
                for nt in range(T // NT):
                    ps = mm_ps.tile([P, NT], F32, tag="mm")
                    for ko in range(DK):
                        nc.tensor.matmul(ps[:], w_sb[:, ko, :],
                                         x_sb[:, ko, ts(nt, NT)],
                                         start=(ko == 0), stop=(ko == DK - 1))
                    nc.scalar.activation(dst[:, ts(nt, NT)], ps[:],
                                         AF.Identity, bias=b_sb[:])

            # ---- projection: v (tokens on partitions) ----
            for tc_i in range(T // P):
                ps = mm_ps.tile([P, NT], F32, tag="mm")
                pv = ps[:, 0:2 * A]
                for ko in range(DK):
                    nc.tensor.matmul(pv, x_sb[:, ko, ts(tc_i, P)], wv_sb[:, ko, :],
                                     start=(ko == 0), stop=False)
                # += ones_col.T @ bv  (adds bias to every token row)
                nc.tensor.matmul(pv, onesc[:], bv_sb[:], start=False, stop=True)
                nc.vector.tensor_copy(out=v_sb[:, tc_i, :, 0:A], in_=pv)

            # ---- attention: flat pipelined stream, heads paired on PE ----
            LOOK = 1  # lookahead in paired-score units (2 psum tiles each)
            stream = []
            for b in range(B):
                for st in range(S // NT):
                    nblk = 4 * st + 4
                    for tb in range(nblk):
                        stream.append((b, st, tb, nblk))

            def issue_scores(i):
                b, st, tb, nblk = stream[i]
                base = b * S
                sps = []
                for h in range(HPC):
                    hs = ds(h * A, A)
                    sp = sc_ps.tile([P, NT], F32, tag="sc", name=f"sp{h}")
                    # h=1 operands live at base partition 64 -> row-group 64;
                    # the two K=64 matmuls execute concurrently on the PE.
                    nc.tensor.matmul(
                        sp[:],
                        kT_sb[hs, base + 128 * tb:base + 128 * (tb + 1)],
                        qT_sb[hs, base + NT * st:base + NT * (st + 1)],
                        start=True, stop=True)
                    sps.append(sp)
                return sps

            # per-batch A2A buffers (token shards of 256)
            SH = S // NCORES  # 256
            a2a_in = [dram.tile([NCORES, P, SH], BF16, name=f"a2a_in{c}")
                      for c in range(B)]
            a2a_out = [dram.tile([NCORES, P, SH], BF16, name=f"a2a_out{c}")
                       for c in range(B)]
            zr_sb = big.tile([P, B, NCORES, SH], BF16)

            def emit_a2a(c):
                for j in range(NCORES):
                    nc.sync.dma_start(a2a_in[c][j],
                                      zT_sb[:, ds(c * S + SH * j, SH)])
                if collective:
                    nc.gpsimd.collective_compute(
                        "AllToAll", mybir.AluOpType.bypass,
                        replica_groups=[list(range(NCORES))],
                        ins=[a2a_in[c].opt()], outs=[a2a_out[c].opt()])
                else:
                    nc.sync.dma_start(a2a_out[c][:], a2a_in[c][:])
                for j in range(NCORES):
                    nc.sync.dma_start(zr_sb[:, c, j, :], a2a_out[c][j])

            zps = [None, None]
            sq = [issue_scores(i) for i in range(min(LOOK, len(stream)))]
            for i, (b, st, tb, nblk) in enumerate(stream):
                if i + LOOK < len(stream):
                    sq.append(issue_scores(i + LOOK))
                sps = sq.pop(0)
                m = tb - 4 * st
                for h in range(HPC):
                    if tb == 0:
                        zps[h] = z_ps.tile([A + 1, NT], F32, tag="z",
                                           name=f"zp{h}")
                    p_sb = work.tile([P, NT], BF16, tag="p", bufs=6)
                    nc.scalar.activation(p_sb[:], sps[h][:], AF.Exp,
                                         scale=SCALE)
                    if m >= 0:  # diagonal block: zero the invalid entries
                        nc.vector.tensor_tensor(
                            out=p_sb[:], in0=p_sb[:], in1=mask_sb[:, m, :],
                            op=mybir.AluOpType.mult)
                    nc.tensor.matmul(
                        zps[h][:], v_sb[:, b * (S // P) + tb, h, :], p_sb[:],
                        start=(tb == 0), stop=(tb == nblk - 1))
                if tb == nblk - 1:
                    base = b * S
                    for h in range(HPC):
                        hs = ds(h * A, A)
                        den = work.tile([1, NT], F32, tag="den")
                        nc.vector.tensor_copy(out=den[:], in_=zps[h][A:A + 1, :])
                        rec = work.tile([1, NT], F32, tag="rec")
                        nc.vector.reciprocal(rec[:], den[:])
                        bc = mm_ps.tile([P, NT], F32, tag="mm", name="bc")
                        nc.tensor.matmul(bc[0:A, :], ones1[:], rec[:],
                                         start=True, stop=True)
                        bc_sb = work.tile([A, NT], F32, tag="bc")
                        nc.vector.tensor_copy(out=bc_sb[:], in_=bc[0:A, :])
                        nc.vector.tensor_tensor(
                            out=zT_sb[hs, base + NT * st:base + NT * (st + 1)],
                            in0=zps[h][0:A, :], in1=bc_sb[:],
                            op=mybir.AluOpType.mult)
                    if st == S // NT - 1:
                        emit_a2a(b)  # batch b fully attended -> exchange

            # ---- output projection per batch shard ----
            for c in range(B):
                for mo in range(DK):
                    ps = mm_ps.tile([P, NT], F32, tag="mm")
                    po = ps[:, 0:SH]
                    for j in range(NCORES):
                        nc.tensor.matmul(po, wo_sb[:, j, mo, :],
                                         zr_sb[:, c, j, :],
                                         start=(j == 0), stop=(j == NCORES - 1))
                    osb = work.tile([P, SH], F32, tag="o")
                    nc.vector.tensor_scalar_add(osb[:], po, bo_sb[:, mo:mo + 1])
                    nc.sync.dma_start(out.ap()[ts(mo, P), ts(c, SH)], osb[:])

    nc.compile()
    return nc


def _prep_inputs(residual, Wq, Wk, Wv, Wo, bq, bk, bv, bo):
    bf = ml_dtypes.bfloat16
    residual = np.asarray(residual, np.float32)
    xT = np.ascontiguousarray(residual.reshape(T, D).T).astype(bf)
    woR = np.ascontiguousarray(
        np.asarray(Wo, np.float32).reshape(NCORES, 2 * A, D)).astype(bf)
    boR = np.ascontiguousarray(np.asarray(bo, np.float32).reshape(DK, P).T)
    # causal masks for the 4 diagonal sub-blocks
    tt = np.arange(P)[:, None]
    ss = np.arange(NT)[None, :]
    mk = np.stack([np.where(128 * m + tt <= ss, 1.0, 0.0)
                   for m in range(4)]).astype(bf)
    in_maps = []
    for i in range(NCORES):
        hsl = slice(HPC * i, HPC * (i + 1))
        wqi = np.ascontiguousarray(
            np.asarray(Wq, np.float32)[hsl].transpose(1, 0, 2).reshape(D, 2 * A)
        ).astype(bf)
        wki = np.ascontiguousarray(
            np.asarray(Wk, np.float32)[hsl].transpose(1, 0, 2).reshape(D, 2 * A)
        ).astype(bf)
        wvi = np.ascontiguousarray(
            np.asarray(Wv, np.float32)[hsl].transpose(1, 0, 2).reshape(D, 2 * A)
        ).astype(bf)
        in_maps.append({
            "xT": xT, "wq": wqi, "wk": wki, "wv": wvi, "woR": woR,
            "bqd": np.asarray(bq, np.float32)[hsl].reshape(2 * A, 1),
            "bkd": np.asarray(bk, np.float32)[hsl].reshape(2 * A, 1),
            "bvd": np.asarray(bv, np.float32)[hsl].reshape(1, 2 * A),
            "bod": boR, "mkd": mk,
        })
    return in_maps


def kernel(residual, Wq, Wk, Wv, Wo, bq, bk, bv, bo, _trace=False):
    if "nc" not in _CACHE:
        _CACHE["nc"] = _build()
    nc = _CACHE["nc"]
    in_maps = _prep_inputs(residual, Wq, Wk, Wv, Wo, bq, bk, bv, bo)
    res = run_bass_kernel_spmd(nc, in_maps, core_ids=list(range(NCORES)),
                               trace=_trace)
    _CACHE["last_result"] = res
    SH = S // NCORES
    full = np.empty((B, S, D), np.float32)
    for i in range(NCORES):
        o = res.results[i]["out"]  # [D, B*SH]
        for c in range(B):
            full[c, SH * i:SH * (i + 1), :] = o[:, c * SH:(c + 1) * SH].T
    return full


# revision 11
# speedup vs baseline: 4.1484x; 1.4134x over previous
"""Distributed causal MHA for Trainium2 (8 NeuronCores).

Problem: B=2, S=2048, D=1024, H=16, A=64 causal attention block.

Sharding: tensor-parallel over heads. Each core owns 2 heads end-to-end
(QKV projection + attention), then an AllToAll exchanges z from
head-sharded to sequence-sharded, and each core computes the output
projection for its 512-token shard (contraction over all 16 heads).
Host concatenates the 8 token shards. No reduction on host.

Layouts (per core):
  xT   [D, T=4096]    residual transposed, tokens b-major (bf16)
  wq/wk/wv [D, 128]   2 local heads packed (h*64+a) (bf16)
  woR  [8, 128, D]    full Wo as (pair j, (h,a), d) (bf16)
  qT/kT [128, T]      (h*64+a) on partitions
  v    [128p=t, 32tc, 2h, 65]  v augmented with ones column (denominator trick)
  zT   [128, T]       attention out, head-packed
  out  [D, 512]       output-projection result for this core's token shard (f32)
"""

import numpy as np
import ml_dtypes

import concourse.bass as bass
import concourse.mybir as mybir
import concourse.tile as tile
from concourse import bacc
from concourse.bass import ts, ds
from concourse.bass_utils import run_bass_kernel_spmd

AF = mybir.ActivationFunctionType
F32 = mybir.dt.float32
BF16 = mybir.dt.bfloat16

B, S, D, H, A = 2, 2048, 1024, 16, 64
NCORES = 8
HPC = H // NCORES          # 2 heads per core
T = B * S                  # 4096 tokens
P = 128
DK = D // P                # 8 contraction chunks
NT = 512                   # free-dim tile for projections / attention
TSH = T // NCORES          # 512 tokens per core shard (output)
SCALE = 1.0 / np.sqrt(A)
NEG = -1.0e9

_CACHE = {}


def _build(collective=True):
    nc = bacc.Bacc("TRN2", target_bir_lowering=False, debug=False,
                   num_devices=NCORES)
    xT = nc.dram_tensor("xT", [D, T], BF16, kind="ExternalInput")
    wq = nc.dram_tensor("wq", [D, 2 * A], BF16, kind="ExternalInput")
    wk = nc.dram_tensor("wk", [D, 2 * A], BF16, kind="ExternalInput")
    wv = nc.dram_tensor("wv", [D, 2 * A], BF16, kind="ExternalInput")
    woR = nc.dram_tensor("woR", [NCORES, 2 * A, D], BF16, kind="ExternalInput")
    bqd = nc.dram_tensor("bqd", [2 * A, 1], F32, kind="ExternalInput")
    bkd = nc.dram_tensor("bkd", [2 * A, 1], F32, kind="ExternalInput")
    bvd = nc.dram_tensor("bvd", [1, 2 * A], F32, kind="ExternalInput")
    bod = nc.dram_tensor("bod", [P, DK], F32, kind="ExternalInput")
    mkd = nc.dram_tensor("mkd", [4, P, NT], BF16, kind="ExternalInput")
    out = nc.dram_tensor("out", [D, TSH], F32, kind="ExternalOutput")

    with tile.TileContext(nc) as tc:
        with tc.tile_pool(name="big", bufs=1) as big, \
             tc.tile_pool(name="work", bufs=3) as work, \
             tc.tile_pool(name="mm_ps", bufs=2, space="PSUM") as mm_ps, \
             tc.tile_pool(name="sc_ps", bufs=4, space="PSUM") as sc_ps, \
             tc.tile_pool(name="z_ps", bufs=2, space="PSUM") as z_ps, \
             tc.tile_pool(name="dram", bufs=1, space="DRAM") as dram:

            # ---- load inputs to SBUF (weights first, x in chunks) ----
            wq_sb = big.tile([P, DK, 2 * A], BF16)
            nc.sync.dma_start(wq_sb[:], wq.ap().rearrange("(ko p) m -> p ko m", p=P))
            wk_sb = big.tile([P, DK, 2 * A], BF16)
            nc.sync.dma_start(wk_sb[:], wk.ap().rearrange("(ko p) m -> p ko m", p=P))
            wv_sb = big.tile([P, DK, 2 * A], BF16)
            nc.sync.dma_start(wv_sb[:], wv.ap().rearrange("(ko p) m -> p ko m", p=P))
            wo_sb = big.tile([P, NCORES, DK, P], BF16)
            nc.sync.dma_start(
                wo_sb[:], woR.ap().rearrange("j p (mo mi) -> p j mo mi", mi=P))
            bq_sb = big.tile([2 * A, 1], F32)
            nc.sync.dma_start(bq_sb[:], bqd[:])
            bk_sb = big.tile([2 * A, 1], F32)
            nc.sync.dma_start(bk_sb[:], bkd[:])
            bv_sb = big.tile([1, 2 * A], F32)
            nc.sync.dma_start(bv_sb[:], bvd[:])
            bo_sb = big.tile([P, DK], F32)
            nc.sync.dma_start(bo_sb[:], bod[:])
            mask_sb = big.tile([P, 4, NT], BF16)
            nc.sync.dma_start(mask_sb[:], mkd.ap().rearrange("m p s -> p m s"))
            x_sb = big.tile([P, DK, T], BF16)
            for tq in range(4):
                for ko in range(DK):
                    nc.sync.dma_start(x_sb[:, ko, ts(tq, T // 4)],
                                      xT.ap()[ts(ko, P), ts(tq, T // 4)])
            ones1 = big.tile([1, A], F32)
            nc.any.memset(ones1[:], 1.0)
            onesc = big.tile([1, P], F32)
            nc.any.memset(onesc[:], 1.0)

            # ---- persistent activations ----
            qT_sb = big.tile([P, T], BF16)     # (h*64+a, token)
            kT_sb = big.tile([P, T], BF16)
            v_sb = big.tile([P, T // P, HPC, A + 1], BF16)  # (t_in, t_chunk, h, a+den)
            nc.any.memset(v_sb[:, :, :, A], 1.0)
            zT_sb = big.tile([P, T], BF16)

            # ---- projection unit emitters (batch-sliced for overlap) ----
            SPB = S // NT   # 4 proj tiles per batch for q/k

            def proj_qk_unit(dst, w_sb, b_sb, nt):
                ps = mm_ps.tile([P, NT], F32, tag="mm", name="pqk")
                for ko in range(DK):
                    nc.tensor.matmul(ps[:], w_sb[:, ko, :],
                                     x_sb[:, ko, ts(nt, NT)],
                                     start=(ko == 0), stop=(ko == DK - 1))
                nc.scalar.activation(dst[:, ts(nt, NT)], ps[:],
                                     AF.Identity, bias=b_sb[:])

            def proj_v_unit(tc_i):
                ps = mm_ps.tile([P, NT], F32, tag="mm", name="pv")
                pv = ps[:, 0:2 * A]
                for ko in range(DK):
                    nc.tensor.matmul(pv, x_sb[:, ko, ts(tc_i, P)],
                                     wv_sb[:, ko, :],
                                     start=(ko == 0), stop=False)
                nc.tensor.matmul(pv, onesc[:], bv_sb[:], start=False, stop=True)
                nc.vector.tensor_copy(out=v_sb[:, tc_i, :, 0:A], in_=pv)

            # batch-0 projections inline (prologue)
            for dst, w_sb, b_sb in ((qT_sb, wq_sb, bq_sb), (kT_sb, wk_sb, bk_sb)):
                for nt in range(SPB):
                    proj_qk_unit(dst, w_sb, b_sb, nt)
            for tc_i in range(S // P):
                proj_v_unit(tc_i)

            # batch-1 projections run as background PE work inside attention
            bg = []
            for dst, w_sb, b_sb in ((qT_sb, wq_sb, bq_sb), (kT_sb, wk_sb, bk_sb)):
                for nt in range(SPB, 2 * SPB):
                    bg.append((lambda dst=dst, w_sb=w_sb, b_sb=b_sb, nt=nt:
                               proj_qk_unit(dst, w_sb, b_sb, nt)))
            for tc_i in range(S // P, T // P):
                bg.append(lambda tc_i=tc_i: proj_v_unit(tc_i))

            # ---- attention: flat pipelined stream, heads paired on PE ----
            LOOK = 1  # lookahead in paired-score units (2 psum tiles each)
            stream = []
            for b in range(B):
                for st in range(S // NT):
                    nblk = 4 * st + 4
                    for tb in range(nblk):
                        stream.append((b, st, tb, nblk))

            def issue_scores(i):
                b, st, tb, nblk = stream[i]
                base = b * S
                sps = []
                for h in range(HPC):
                    hs = ds(h * A, A)
                    sp = sc_ps.tile([P, NT], F32, tag="sc", name=f"sp{h}")
                    # h=1 operands live at base partition 64 -> row-group 64;
                    # the two K=64 matmuls execute concurrently on the PE.
                    nc.tensor.matmul(
                        sp[:],
                        kT_sb[hs, base + 128 * tb:base + 128 * (tb + 1)],
                        qT_sb[hs, base + NT * st:base + NT * (st + 1)],
                        start=True, stop=True)
                    sps.append(sp)
                return sps

            # per-batch A2A buffers (token shards of 256)
            SH = S // NCORES  # 256
            a2a_in = [dram.tile([NCORES, P, SH], BF16, name=f"a2a_in{c}")
                      for c in range(B)]
            a2a_out = [dram.tile([NCORES, P, SH], BF16, name=f"a2a_out{c}")
                       for c in range(B)]
            zr_sb = big.tile([P, B, NCORES, SH], BF16)

            def emit_a2a(c):
                for j in range(NCORES):
                    nc.sync.dma_start(a2a_in[c][j],
                                      zT_sb[:, ds(c * S + SH * j, SH)])
                if collective:
                    nc.gpsimd.collective_compute(
                        "AllToAll", mybir.AluOpType.bypass,
                        replica_groups=[list(range(NCORES))],
                        ins=[a2a_in[c].opt()], outs=[a2a_out[c].opt()])
                else:
                    nc.sync.dma_start(a2a_out[c][:], a2a_in[c][:])
                for j in range(NCORES):
                    nc.sync.dma_start(zr_sb[:, c, j, :], a2a_out[c][j])

            def outproj_unit(c, mo):
                ps = mm_ps.tile([P, NT], F32, tag="mm", name="po")
                po = ps[:, 0:SH]
                for j in range(NCORES):
                    nc.tensor.matmul(po, wo_sb[:, j, mo, :], zr_sb[:, c, j, :],
                                     start=(j == 0), stop=(j == NCORES - 1))
                osb = work.tile([P, SH], F32, tag="o")
                nc.vector.tensor_scalar_add(osb[:], po, bo_sb[:, mo:mo + 1])
                nc.sync.dma_start(out.ap()[ts(mo, P), ts(c, SH)], osb[:])

            bg2 = []   # out-proj c=0 units, gated to late b=1 attention
            BG2_GATE = int(len(stream) * 0.78)
            zps = [None, None]
            sq = [issue_scores(i) for i in range(min(LOOK, len(stream)))]
            for i, (b, st, tb, nblk) in enumerate(stream):
                if i + LOOK < len(stream):
                    sq.append(issue_scores(i + LOOK))
                if bg:
                    bg.pop(0)()
                elif bg2 and i >= BG2_GATE:
                    bg2.pop(0)()
                sps = sq.pop(0)
                m = tb - 4 * st
                for h in range(HPC):
                    if tb == 0:
                        zps[h] = z_ps.tile([A + 1, NT], F32, tag="z",
                                           name=f"zp{h}")
                    p_sb = work.tile([P, NT], BF16, tag="p", bufs=6)
                    nc.scalar.activation(p_sb[:], sps[h][:], AF.Exp,
                                         scale=SCALE)
                    if m >= 0:  # diagonal block: zero the invalid entries
                        nc.vector.tensor_tensor(
                            out=p_sb[:], in0=p_sb[:], in1=mask_sb[:, m, :],
                            op=mybir.AluOpType.mult)
                    nc.tensor.matmul(
                        zps[h][:], v_sb[:, b * (S // P) + tb, h, :], p_sb[:],
                        start=(tb == 0), stop=(tb == nblk - 1))
                if tb == nblk - 1:
                    base = b * S
                    for h in range(HPC):
                        hs = ds(h * A, A)
                        den = work.tile([1, NT], F32, tag="den")
                        nc.vector.tensor_copy(out=den[:], in_=zps[h][A:A + 1, :])
                        rec = work.tile([1, NT], F32, tag="rec")
                        nc.vector.reciprocal(rec[:], den[:])
                        bc = mm_ps.tile([P, NT], F32, tag="mm", name="bc")
                        nc.tensor.matmul(bc[0:A, :], ones1[:], rec[:],
                                         start=True, stop=True)
                        bc_sb = work.tile([A, NT], F32, tag="bc")
                        nc.vector.tensor_copy(out=bc_sb[:], in_=bc[0:A, :])
                        nc.vector.tensor_tensor(
                            out=zT_sb[hs, base + NT * st:base + NT * (st + 1)],
                            in0=zps[h][0:A, :], in1=bc_sb[:],
                            op=mybir.AluOpType.mult)
                    if st == S // NT - 1:
                        emit_a2a(b)  # batch b fully attended -> exchange
                        if b == 0:
                            bg2.extend([
                                (lambda mo=mo: outproj_unit(0, mo))
                                for mo in range(DK)])

            # ---- remaining output projection (c=1 + any leftovers) ----
            while bg:
                bg.pop(0)()
            while bg2:
                bg2.pop(0)()
            for mo in range(DK):
                outproj_unit(1, mo)

    nc.compile()
    return nc


def _prep_inputs(residual, Wq, Wk, Wv, Wo, bq, bk, bv, bo):
    bf = ml_dtypes.bfloat16
    residual = np.asarray(residual, np.float32)
    xT = np.ascontiguousarray(residual.reshape(T, D).T).astype(bf)
    woR = np.ascontiguousarray(
        np.asarray(Wo, np.float32).reshape(NCORES, 2 * A, D)).astype(bf)
    boR = np.ascontiguousarray(np.asarray(bo, np.float32).reshape(DK, P).T)
    # causal masks for the 4 diagonal sub-blocks
    tt = np.arange(P)[:, None]
    ss = np.arange(NT)[None, :]
    mk = np.stack([np.where(128 * m + tt <= ss, 1.0, 0.0)
                   for m in range(4)]).astype(bf)
    in_maps = []
    for i in range(NCORES):
        hsl = slice(HPC * i, HPC * (i + 1))
        wqi = np.ascontiguousarray(
            np.asarray(Wq, np.float32)[hsl].transpose(1, 0, 2).reshape(D, 2 * A)
        ).astype(bf)
        wki = np.ascontiguousarray(
            np.asarray(Wk, np.float32)[hsl].transpose(1, 0, 2).reshape(D, 2 * A)
        ).astype(bf)
        wvi = np.ascontiguousarray(
            np.asarray(Wv, np.float32)[hsl].transpose(1, 0, 2).reshape(D, 2 * A)
        ).astype(bf)
        in_maps.append({
            "xT": xT, "wq": wqi, "wk": wki, "wv": wvi, "woR": woR,
            "bqd": np.asarray(bq, np.float32)[hsl].reshape(2 * A, 1),
            "bkd": np.asarray(bk, np.float32)[hsl].reshape(2 * A, 1),
            "bvd": np.asarray(bv, np.float32)[hsl].reshape(1, 2 * A),
            "bod": boR, "mkd": mk,
        })
    return in_maps


def kernel(residual, Wq, Wk, Wv, Wo, bq, bk, bv, bo, _trace=False):
    if "nc" not in _CACHE:
        _CACHE["nc"] = _build()
    nc = _CACHE["nc"]
    in_maps = _prep_inputs(residual, Wq, Wk, Wv, Wo, bq, bk, bv, bo)
    res = run_bass_kernel_spmd(nc, in_maps, core_ids=list(range(NCORES)),
                               trace=_trace)
    _CACHE["last_result"] = res
    SH = S // NCORES
    full = np.empty((B, S, D), np.float32)
    for i in range(NCORES):
        o = res.results[i]["out"]  # [D, B*SH]
        for c in range(B):
            full[c, SH * i:SH * (i + 1), :] = o[:, c * SH:(c + 1) * SH].T
    return full


# revision 15
# speedup vs baseline: 5.5602x; 1.3403x over previous
"""Distributed causal MHA for Trainium2 (8 NeuronCores).

Problem: B=2, S=2048, D=1024, H=16, A=64 causal attention block.

Sharding: tensor-parallel over heads. Each core owns 2 heads end-to-end
(QKV projection + attention), then an AllToAll exchanges z from
head-sharded to sequence-sharded, and each core computes the output
projection for its 512-token shard (contraction over all 16 heads).
Host concatenates the 8 token shards. No reduction on host.

Layouts (per core):
  xT   [D, T=4096]    residual transposed, tokens b-major (bf16)
  wq/wk/wv [D, 128]   2 local heads packed (h*64+a) (bf16)
  woR  [8, 128, D]    full Wo as (pair j, (h,a), d) (bf16)
  qT/kT [128, T]      (h*64+a) on partitions
  v    [128p=t, 32tc, 2h, 65]  v augmented with ones column (denominator trick)
  zT   [128, T]       attention out, head-packed
  out  [D, 512]       output-projection result for this core's token shard (f32)
"""

import numpy as np
import ml_dtypes

import concourse.bass as bass
import concourse.mybir as mybir
import concourse.tile as tile
from concourse import bacc
from concourse.bass import ts, ds
from concourse.bass_utils import run_bass_kernel_spmd

AF = mybir.ActivationFunctionType
F32 = mybir.dt.float32
BF16 = mybir.dt.bfloat16

B, S, D, H, A = 2, 2048, 1024, 16, 64
NCORES = 8
HPC = H // NCORES          # 2 heads per core
T = B * S                  # 4096 tokens
P = 128
DK = D // P                # 8 contraction chunks
NT = 512                   # free-dim tile for projections / attention
TSH = T // NCORES          # 512 tokens per core shard (output)
SCALE = 1.0 / np.sqrt(A)
NEG = -1.0e9

_CACHE = {}


def _build(collective=True):
    nc = bacc.Bacc("TRN2", target_bir_lowering=False, debug=False,
                   num_devices=NCORES)
    xT = nc.dram_tensor("xT", [D, T], BF16, kind="ExternalInput")
    wq = nc.dram_tensor("wq", [D, 2 * A], BF16, kind="ExternalInput")
    wk = nc.dram_tensor("wk", [D, 2 * A], BF16, kind="ExternalInput")
    wv = nc.dram_tensor("wv", [D, 2 * A], BF16, kind="ExternalInput")
    woR = nc.dram_tensor("woR", [NCORES, 2 * A, D], BF16, kind="ExternalInput")
    bqd = nc.dram_tensor("bqd", [2 * A, 1], F32, kind="ExternalInput")
    bkd = nc.dram_tensor("bkd", [2 * A, 1], F32, kind="ExternalInput")
    bvd = nc.dram_tensor("bvd", [1, 2 * A], F32, kind="ExternalInput")
    bod = nc.dram_tensor("bod", [P, DK], F32, kind="ExternalInput")
    mkd = nc.dram_tensor("mkd", [4, P, NT], BF16, kind="ExternalInput")
    out = nc.dram_tensor("out", [D, TSH], F32, kind="ExternalOutput")

    with tile.TileContext(nc) as tc:
        with tc.tile_pool(name="big", bufs=1) as big, \
             tc.tile_pool(name="work", bufs=3) as work, \
             tc.tile_pool(name="mm_ps", bufs=2, space="PSUM") as mm_ps, \
             tc.tile_pool(name="sc_ps", bufs=4, space="PSUM") as sc_ps, \
             tc.tile_pool(name="z_ps", bufs=2, space="PSUM") as z_ps, \
             tc.tile_pool(name="dram", bufs=1, space="DRAM") as dram:

            # ---- load inputs to SBUF (weights first, x in chunks) ----
            wq_sb = big.tile([P, DK, 2 * A], BF16)
            nc.sync.dma_start(wq_sb[:], wq.ap().rearrange("(ko p) m -> p ko m", p=P))
            wk_sb = big.tile([P, DK, 2 * A], BF16)
            nc.sync.dma_start(wk_sb[:], wk.ap().rearrange("(ko p) m -> p ko m", p=P))
            wv_sb = big.tile([P, DK, 2 * A], BF16)
            nc.sync.dma_start(wv_sb[:], wv.ap().rearrange("(ko p) m -> p ko m", p=P))
            wo_sb = big.tile([P, NCORES, DK, P], BF16)
            nc.sync.dma_start(
                wo_sb[:], woR.ap().rearrange("j p (mo mi) -> p j mo mi", mi=P))
            bq_sb = big.tile([2 * A, 1], F32)
            nc.sync.dma_start(bq_sb[:], bqd[:])
            bk_sb = big.tile([2 * A, 1], F32)
            nc.sync.dma_start(bk_sb[:], bkd[:])
            bv_sb = big.tile([1, 2 * A], F32)
            nc.sync.dma_start(bv_sb[:], bvd[:])
            bo_sb = big.tile([P, DK], F32)
            nc.sync.dma_start(bo_sb[:], bod[:])
            mask_sb = big.tile([P, 4, NT], BF16)
            nc.sync.dma_start(mask_sb[:], mkd.ap().rearrange("m p s -> p m s"))
            x_sb = big.tile([P, DK, T], BF16)
            for tq in range(4):
                for ko in range(DK):
                    nc.sync.dma_start(x_sb[:, ko, ts(tq, T // 4)],
                                      xT.ap()[ts(ko, P), ts(tq, T // 4)])
            ones1 = big.tile([1, A], F32)
            nc.any.memset(ones1[:], 1.0)
            onesc = big.tile([1, P], F32)
            nc.any.memset(onesc[:], 1.0)

            # ---- persistent activations ----
            qT_sb = big.tile([P, T], BF16)     # (h*64+a, token)
            kT_sb = big.tile([P, T], BF16)
            v_sb = big.tile([P, T // P, HPC, A + 1], BF16)  # (t_in, t_chunk, h, a+den)
            nc.any.memset(v_sb[:, :, :, A], 1.0)
            zT_sb = big.tile([P, T], BF16)

            # ---- projection unit emitters (batch-sliced for overlap) ----
            SPB = S // NT   # 4 proj tiles per batch for q/k

            def proj_qk_unit(dst, w_sb, b_sb, nt):
                ps = mm_ps.tile([P, NT], F32, tag="mm", name="pqk")
                for ko in range(DK):
                    nc.tensor.matmul(ps[:], w_sb[:, ko, :],
                                     x_sb[:, ko, ts(nt, NT)],
                                     start=(ko == 0), stop=(ko == DK - 1))
                nc.scalar.activation(dst[:, ts(nt, NT)], ps[:],
                                     AF.Identity, bias=b_sb[:])

            def proj_v_unit(tc_i):
                ps = mm_ps.tile([P, NT], F32, tag="mm", name="pv")
                pv = ps[:, 0:2 * A]
                for ko in range(DK):
                    nc.tensor.matmul(pv, x_sb[:, ko, ts(tc_i, P)],
                                     wv_sb[:, ko, :],
                                     start=(ko == 0), stop=False)
                nc.tensor.matmul(pv, onesc[:], bv_sb[:], start=False, stop=True)
                nc.vector.tensor_copy(out=v_sb[:, tc_i, :, 0:A], in_=pv)

            # batch-0 projections inline (prologue)
            for dst, w_sb, b_sb in ((qT_sb, wq_sb, bq_sb), (kT_sb, wk_sb, bk_sb)):
                for nt in range(SPB):
                    proj_qk_unit(dst, w_sb, b_sb, nt)
            for tc_i in range(S // P):
                proj_v_unit(tc_i)

            # batch-1 projections run as background PE work inside attention
            bg = []
            for dst, w_sb, b_sb in ((qT_sb, wq_sb, bq_sb), (kT_sb, wk_sb, bk_sb)):
                for nt in range(SPB, 2 * SPB):
                    bg.append((lambda dst=dst, w_sb=w_sb, b_sb=b_sb, nt=nt:
                               proj_qk_unit(dst, w_sb, b_sb, nt)))
            for tc_i in range(S // P, T // P):
                bg.append(lambda tc_i=tc_i: proj_v_unit(tc_i))

            # ---- attention: flat pipelined stream, heads paired on PE ----
            LOOK = 1  # lookahead in paired-score units (2 psum tiles each)
            stream = []
            for b in range(B):
                for st in range(S // NT):
                    nblk = 4 * st + 4
                    for tb in range(nblk):
                        stream.append((b, st, tb, nblk))

            def issue_scores(i):
                b, st, tb, nblk = stream[i]
                base = b * S
                sps = []
                for h in range(HPC):
                    hs = ds(h * A, A)
                    sp = sc_ps.tile([P, NT], F32, tag="sc", name=f"sp{h}")
                    # h=1 operands live at base partition 64 -> row-group 64;
                    # the two K=64 matmuls execute concurrently on the PE.
                    nc.tensor.matmul(
                        sp[:],
                        kT_sb[hs, base + 128 * tb:base + 128 * (tb + 1)],
                        qT_sb[hs, base + NT * st:base + NT * (st + 1)],
                        start=True, stop=True)
                    sps.append(sp)
                return sps

            # per-batch A2A buffers (token shards of 256)
            SH = S // NCORES  # 256
            a2a_in = [dram.tile([NCORES, P, SH], BF16, name=f"a2a_in{c}")
                      for c in range(B)]
            a2a_out = [dram.tile([NCORES, P, SH], BF16, name=f"a2a_out{c}")
                       for c in range(B)]
            zr_sb = big.tile([P, B, NCORES, SH], BF16)

            def emit_a2a(c):
                for j in range(NCORES):
                    nc.sync.dma_start(a2a_in[c][j],
                                      zT_sb[:, ds(c * S + SH * j, SH)])
                if collective:
                    nc.gpsimd.collective_compute(
                        "AllToAll", mybir.AluOpType.bypass,
                        replica_groups=[list(range(NCORES))],
                        ins=[a2a_in[c].opt()], outs=[a2a_out[c].opt()])
                else:
                    nc.sync.dma_start(a2a_out[c][:], a2a_in[c][:])
                for j in range(NCORES):
                    nc.sync.dma_start(zr_sb[:, c, j, :], a2a_out[c][j])

            def outproj_unit(c, mo):
                ps = mm_ps.tile([P, NT], F32, tag="mm", name="po")
                po = ps[:, 0:SH]
                for j in range(NCORES):
                    nc.tensor.matmul(po, wo_sb[:, j, mo, :], zr_sb[:, c, j, :],
                                     start=(j == 0), stop=(j == NCORES - 1))
                osb = work.tile([P, SH], F32, tag="o")
                nc.vector.tensor_scalar_add(osb[:], po, bo_sb[:, mo:mo + 1])
                nc.sync.dma_start(out.ap()[ts(mo, P), ts(c, SH)], osb[:])

            bg2 = []   # out-proj c=0 units, gated to late b=1 attention
            BG2_GATE = int(len(stream) * 0.78)
            zps = [None, None]
            sq = [issue_scores(i) for i in range(min(LOOK, len(stream)))]
            for i, (b, st, tb, nblk) in enumerate(stream):
                if i + LOOK < len(stream):
                    sq.append(issue_scores(i + LOOK))
                if bg:
                    bg.pop(0)()
                elif bg2 and i >= BG2_GATE:
                    bg2.pop(0)()
                sps = sq.pop(0)
                m = tb - 4 * st
                for h in range(HPC):
                    if tb == 0:
                        zps[h] = z_ps.tile([A + 1, NT], F32, tag="z",
                                           name=f"zp{h}")
                    p_sb = work.tile([P, NT], BF16, tag="p", bufs=6)
                    nc.scalar.activation(p_sb[:], sps[h][:], AF.Exp,
                                         scale=SCALE)
                    if m >= 0:  # diagonal block: zero the invalid entries
                        nc.vector.tensor_tensor(
                            out=p_sb[:], in0=p_sb[:], in1=mask_sb[:, m, :],
                            op=mybir.AluOpType.mult)
                    nc.tensor.matmul(
                        zps[h][:], v_sb[:, b * (S // P) + tb, h, :], p_sb[:],
                        start=(tb == 0), stop=(tb == nblk - 1))
                if tb == nblk - 1:
                    base = b * S
                    for h in range(HPC):
                        hs = ds(h * A, A)
                        den = work.tile([1, NT], F32, tag="den")
                        nc.vector.tensor_copy(out=den[:], in_=zps[h][A:A + 1, :])
                        rec = work.tile([1, NT], F32, tag="rec")
                        nc.vector.reciprocal(rec[:], den[:])
                        bc = mm_ps.tile([P, NT], F32, tag="mm", name="bc")
                        nc.tensor.matmul(bc[0:A, :], ones1[:], rec[:],
                                         start=True, stop=True)
                        bc_sb = work.tile([A, NT], F32, tag="bc")
                        nc.vector.tensor_copy(out=bc_sb[:], in_=bc[0:A, :])
                        nc.vector.tensor_tensor(
                            out=zT_sb[hs, base + NT * st:base + NT * (st + 1)],
                            in0=zps[h][0:A, :], in1=bc_sb[:],
                            op=mybir.AluOpType.mult)
                    if st == S // NT - 1:
                        emit_a2a(b)  # batch b fully attended -> exchange
                        if b == 0:
                            bg2.extend([
                                (lambda mo=mo: outproj_unit(0, mo))
                                for mo in range(DK)])

            # ---- remaining output projection (c=1 + any leftovers) ----
            while bg:
                bg.pop(0)()
            while bg2:
                bg2.pop(0)()
            for mo in range(DK):
                outproj_unit(1, mo)

    nc.compile()
    return nc


def _prep_inputs(residual, Wq, Wk, Wv, Wo, bq, bk, bv, bo):
    bf = ml_dtypes.bfloat16
    residual = np.asarray(residual, np.float32)
    xT = np.ascontiguousarray(residual.reshape(T, D).T).astype(bf)
    woR = np.ascontiguousarray(
        np.asarray(Wo, np.float32).reshape(NCORES, 2 * A, D)).astype(bf)
    boR = np.ascontiguousarray(np.asarray(bo, np.float32).reshape(DK, P).T)
    # causal masks for the 4 diagonal sub-blocks
    tt = np.arange(P)[:, None]
    ss = np.arange(NT)[None, :]
    mk = np.stack([np.where(128 * m + tt <= ss, 1.0, 0.0)
                   for m in range(4)]).astype(bf)
    in_maps = []
    for i in range(NCORES):
        hsl = slice(HPC * i, HPC * (i + 1))
        wqi = np.ascontiguousarray(
            np.asarray(Wq, np.float32)[hsl].transpose(1, 0, 2).reshape(D, 2 * A)
        ).astype(bf)
        wki = np.ascontiguousarray(
            np.asarray(Wk, np.float32)[hsl].transpose(1, 0, 2).reshape(D, 2 * A)
        ).astype(bf)
        wvi = np.ascontiguousarray(
            np.asarray(Wv, np.float32)[hsl].transpose(1, 0, 2).reshape(D, 2 * A)
        ).astype(bf)
        in_maps.append({
            "xT": xT, "wq": wqi, "wk": wki, "wv": wvi, "woR": woR,
            "bqd": np.asarray(bq, np.float32)[hsl].reshape(2 * A, 1),
            "bkd": np.asarray(bk, np.float32)[hsl].reshape(2 * A, 1),
            "bvd": np.asarray(bv, np.float32)[hsl].reshape(1, 2 * A),
            "bod": boR, "mkd": mk,
        })
    return in_maps


def kernel(residual, Wq, Wk, Wv, Wo, bq, bk, bv, bo, _trace=False):
    if "nc" not in _CACHE:
        _CACHE["nc"] = _build()
    nc = _CACHE["nc"]
    in_maps = _prep_inputs(residual, Wq, Wk, Wv, Wo, bq, bk, bv, bo)
    res = run_bass_kernel_spmd(nc, in_maps, core_ids=list(range(NCORES)),
                               trace=_trace)
    _CACHE["last_result"] = res
    SH = S // NCORES
    full = np.empty((B, S, D), np.float32)
    for i in range(NCORES):
        o = res.results[i]["out"]  # [D, B*SH]
        for c in range(B):
            full[c, SH * i:SH * (i + 1), :] = o[:, c * SH:(c + 1) * SH].T
    return full
